# revision 7
# baseline (speedup 1.0000x reference)
"""BiModal attention kernel for Trainium2 (8 NeuronCores, data-parallel over batch).

Per core (one batch b): x, y: [2048, 128] fp32.
  S = x @ y.T                    (f32r matmuls, [2048, 2048])
  E = exp(S)                     (unshifted; softmax is shift-invariant and
                                  |S| <~ 67 so exp stays in fp32/bf16 range)
  a1 = (E @ y) / rowsum(E) * x
  a2 = (E.T @ x) / colsum(E) * y
  out = concat([a1, a2], -1)     ([2048, 256])

Layout: rows are relabeled r = 16*p + b (p = SBUF partition, b = block index)
so every DRAM transfer is contiguous per partition; applied consistently to s
and t everywhere, so the math is unchanged.

Pipeline (exp on ACT is the 40us spine; everything else hides under it):
  head:    x/y DMA in; PE-transposes build xT/yT (exact f32, HAM warmup)
  panel 0: S(:, :1024) -> exp(+l1 accum) -> paired ET xbar issue (Sync);
           o2 cols :1024 accumulate (lag 2)
  panel 1: S(:, 1024:) -> exp -> ET pairs; o2 cols 1024: accumulate;
           o1 first-half (t 0:1024) accumulation in 2 rotating banks;
           l2 colsums of t-blocks 0..7 (fused tensor_tensor_reduce)
  final:   o1 second half + drain-adds; all gating epilogues, pipelined.
ET is stored pair-interleaved ([tp, pair, i, tb, sp]) so each xbar transpose
moves a contiguous [128, 2048] block; s-chunk order seen by o1 is unchanged.
"""
import sys

sys.path.insert(0, "/opt/trn_rl_repo")

import os
import numpy as np

import concourse.bass as bass
import concourse.mybir as mybir
import concourse.tile as tile
from concourse.tile_rust import add_dep_helper
from concourse import bacc
from concourse.bass_utils import run_bass_kernel_spmd
from concourse.masks import make_identity

f32 = mybir.dt.float32
f32r = mybir.dt.float32r
bf16 = mybir.dt.bfloat16

B = 8
S = 2048
D = 128
P = 128
NB = S // P          # 16 blocks
NP = 2               # panels
PW = S // NP         # panel width (1024)
PB = PW // P         # t-blocks per panel (8)
KP = PB              # transpose pairs per panel (8)

_NC_CACHE = None
LAST_EXEC_NS = None


def _build_program(nc):
    x_d = nc.dram_tensor("x", [S, D], f32, kind="ExternalInput").ap()
    y_d = nc.dram_tensor("y", [S, D], f32, kind="ExternalInput").ap()
    out_d = nc.dram_tensor("out", [S, 2 * D], f32, kind="ExternalOutput").ap()

    x_dv = x_d.rearrange("(p b) d -> p b d", p=P)      # [128, 16, 128]
    y_dv = y_d.rearrange("(p b) d -> p b d", p=P)
    out_dv = out_d.rearrange("(p b) c -> p b c", p=P)  # [128, 16, 256]

    Exp = mybir.ActivationFunctionType.Exp
    Copy = mybir.ActivationFunctionType.Copy
    MUL = mybir.AluOpType.mult
    ADD = mybir.AluOpType.add
    AX = mybir.AxisListType.X

    with tile.TileContext(nc) as tc:
        with (
            tc.tile_pool(name="sb", bufs=1) as sb,
            tc.tile_pool(name="stg", bufs=2) as stg,
            tc.tile_pool(name="ps", bufs=1, space="PSUM") as ps,
        ):
            # ---- persistent SBUF tensors ----
            x_sb = sb.tile([P, NB, D], f32, tag="x_sb")
            y_sb = sb.tile([P, NB, D], f32, tag="y_sb")
            xT = sb.tile([P, NB, P], f32r, tag="xT")       # [d, sb, sp]
            yT = sb.tile([P, NB, P], f32r, tag="yT")       # [d, tb, tp]
            x_hi = sb.tile([P, NB, D], bf16, tag="x_hi")
            y_hi = sb.tile([P, NB, D], bf16, tag="y_hi")
            E0 = sb.tile([P, NB, PW], bf16, tag="E0")      # [sp, sb, t<1024]
            E1 = sb.tile([P, NB, PW], bf16, tag="E1")      # [sp, sb, t-1024]
            # pair-interleaved transposed E: [tp, pair k, i in pair, tbl, sp]
            ET0 = sb.tile([P, KP, 2, PB, P], bf16, tag="ET0")
            ET1 = sb.tile([P, KP, 2, PB, P], bf16, tag="ET1")
            o1T = sb.tile([P, S], f32, tag="o1T")          # [d, s]
            o2T = sb.tile([P, S], f32, tag="o2T")          # [d, t]
            ident = sb.tile([P, P], f32, tag="ident")
            l1p = sb.tile([P, NB, 2], f32, tag="l1p")
            l1 = sb.tile([P, NB], f32, tag="l1")
            r1 = sb.tile([P, NB], f32, tag="r1")
            l2a = sb.tile([P, PB], f32, tag="l2a")
            l2 = sb.tile([P, NB], f32, tag="l2")
            r2 = sb.tile([P, NB], f32, tag="r2")
            ttr_o = sb.tile([P, PW], bf16, tag="ttr_o")    # ttr scratch
            sc2 = sb.tile([P, PB, D], f32, tag="sc2")      # epi2 ACT scale out

            # PSUM: A0/A1 [P,1024] = S rotation -> final epi rotation
            #       B0/B1 [P,512]  = o2 q0/q1 -> o1 group rotation
            #       C0/C1 [P,512]  = transpose staging -> o2 q2/q3 -> epi rot
            sA = [ps.tile([P, PW], f32, tag="A0", name="sA0"),
                  ps.tile([P, PW], f32, tag="A1", name="sA1")]
            oB = [ps.tile([P, 512], f32, tag="B0", name="oB0"),
                  ps.tile([P, 512], f32, tag="B1", name="oB1")]
            oC = [ps.tile([P, 512], f32, tag="C0", name="oC0"),
                  ps.tile([P, 512], f32, tag="C1", name="oC1")]

            make_identity(nc, ident[:])

            # ---- input DMA (Sync queue; ET pair issues follow) ----
            nc.sync.dma_start(y_sb[:, 0:4], y_dv[:, 0:4])
            nc.sync.dma_start(x_sb[:, 0:4], x_dv[:, 0:4])
            nc.sync.dma_start(y_sb[:, 4:8], y_dv[:, 4:8])
            nc.sync.dma_start(x_sb[:, 4:8], x_dv[:, 4:8])
            nc.sync.dma_start(y_sb[:, 8:16], y_dv[:, 8:16])
            nc.sync.dma_start(x_sb[:, 8:16], x_dv[:, 8:16])

            # ---- head: PE transposes (warmup + exact f32 xT/yT) ----
            ntr = [0]

            def block_T(which, b, eng):
                v_sb, vT = (x_sb, xT) if which == "x" else (y_sb, yT)
                k = ntr[0] % 2
                ntr[0] += 1
                tr = nc.tensor.transpose(oC[k][:, 0:P], v_sb[:, b, :], ident[:])
                if eng is nc.scalar:
                    nc.scalar.copy(vT[:, b, :], oC[k][:, 0:P])
                else:
                    nc.vector.tensor_copy(vT[:, b, :], oC[k][:, 0:P])
                return tr

            for which, b in [("y", 0), ("y", 1), ("y", 2), ("y", 3), ("x", 0)]:
                block_T(which, b, nc.scalar)
            for which, b in [("y", 4), ("y", 5), ("y", 6), ("y", 7),
                             ("x", 1), ("x", 2), ("x", 3)]:
                block_T(which, b, nc.vector)
            rest_list = ([("x", b) for b in range(4, 16)]
                         + [("y", b) for b in range(8, 16)])

            nc.vector.tensor_copy(x_hi[:, 0:8], x_sb[:, 0:8])

            yT_f = yT[:].rearrange("p b d -> p (b d)")
            E_t = [E0, E1]
            ET_t = [ET0, ET1]

            def et_pair(ct, k):
                # transpose E rows i=2k,2k+1 (panel ct) into ET pair slot k
                # out 3D = [dst partition (t), column-block (i, tbl), src
                # partition (s)] — matches the xbar descriptor layout
                nc.sync.dma_start_transpose(
                    ET_t[ct][:, k].rearrange("p a b c -> p (a b) c"),
                    E_t[ct][:, 2 * k:2 * k + 2, :].rearrange("p a b -> p (a b)"))

            def o1_mov(tb, q):
                return ET_t[tb // PB][:, 2 * q:2 * q + 2, :, tb % PB, :]

            # ---- panel 0 ----
            for i in range(NB):
                slot = sA[i % 2][:]
                nc.tensor.matmul(slot[:, 0:512], xT[:, i, :],
                                 yT_f[:, 0:512], start=True, stop=True)
                sm = nc.tensor.matmul(slot[:, 512:1024], xT[:, i, :],
                                      yT_f[:, 512:1024], start=True, stop=True)
                if i >= 2:
                    for _ in range(2):
                        if rest_list:
                            tr = block_T(*rest_list.pop(0), nc.vector)
                            add_dep_helper(tr.ins, sm.ins, sync=False,
                                           reason="keep transpose at slot")
                    j = i - 2
                    for q in range(2):
                        om = nc.tensor.matmul(oB[q][:], x_hi[:, j, :],
                                              E0[:, j, q * 512:(q + 1) * 512],
                                              start=(j == 0), stop=(j == NB - 1))
                        if q == 0:
                            add_dep_helper(om.ins, sm.ins, sync=False,
                                           reason="keep o2 at slot")
                nc.scalar.activation(E0[:, i, :], slot, Exp,
                                     accum_out=l1p[:, i, 0:1])
                if i % 2 == 1:
                    et_pair(0, i // 2)
                if i == 4:
                    nc.vector.tensor_copy(x_hi[:, 8:16], x_sb[:, 8:16])
                if i == 6:
                    nc.vector.tensor_copy(y_hi[:, 0:8], y_sb[:, 0:8])
                if i == 8:
                    nc.vector.tensor_copy(y_hi[:, 8:16], y_sb[:, 8:16])
            for j in (14, 15):
                for q in range(2):
                    nc.tensor.matmul(oB[q][:], x_hi[:, j, :],
                                     E0[:, j, q * 512:(q + 1) * 512],
                                     start=False, stop=(j == NB - 1))

            # ---- panel 1 ----
            for i in range(NB):
                slot = sA[i % 2][:]
                nc.tensor.matmul(slot[:, 0:512], xT[:, i, :],
                                 yT_f[:, 1024:1536], start=True, stop=True)
                sm = nc.tensor.matmul(slot[:, 512:1024], xT[:, i, :],
                                      yT_f[:, 1536:2048], start=True, stop=True)
                if i == 0:
                    # drain o2 cols 0:1024; B banks go to o1 groups
                    nc.vector.tensor_copy(o2T[:, 0:512], oB[0][:])
                    nc.vector.tensor_copy(o2T[:, 512:1024], oB[1][:])
                # o1 first half: group q = i//4 accumulates t-blocks 0..7
                q = i // 4
                for z in range(2):
                    tb = 2 * (i % 4) + z
                    o1m = nc.tensor.matmul(oB[q % 2][:], y_hi[:, tb, :],
                                           o1_mov(tb, q),
                                           start=(tb == 0), stop=(tb == PB - 1))
                    if z == 0:
                        add_dep_helper(o1m.ins, sm.ins, sync=False,
                                       reason="keep o1 at slot")
                if i % 4 == 0 and i >= 4:
                    # previous o1 group finished last iter -> drain it
                    pq = q - 1
                    nc.vector.tensor_copy(o1T[:, pq * 512:(pq + 1) * 512],
                                          oB[pq % 2][:])
                if i >= 2:
                    j = i - 2
                    for qq in range(2):
                        om = nc.tensor.matmul(oC[qq][:], x_hi[:, j, :],
                                              E1[:, j, qq * 512:(qq + 1) * 512],
                                              start=(j == 0), stop=(j == NB - 1))
                        if qq == 0:
                            add_dep_helper(om.ins, sm.ins, sync=False,
                                           reason="keep o2 at slot")
                nc.scalar.activation(E1[:, i, :], slot, Exp,
                                     accum_out=l1p[:, i, 1:2])
                if i % 2 == 1:
                    et_pair(1, i // 2)
                # l2 for t-blocks 0..7 (full colsum, one fused op each)
                if 1 <= i < 9:
                    tb = i - 1
                    nc.vector.tensor_tensor_reduce(
                        ttr_o[:], ET0[:, 0:4, :, tb, :], ET0[:, 4:8, :, tb, :],
                        1.0, 0.0, op0=ADD, op1=ADD,
                        accum_out=l2[:, tb:tb + 1])
                if i == 9:
                    nc.vector.reciprocal(r2[:, 0:8], l2[:, 0:8])
                # first-half colsums for t-blocks 8..15 (pairs 0..3 of ET1)
                if i >= 12:
                    for z in range(2):
                        tbl = 2 * (i - 12) + z
                        nc.vector.tensor_tensor_reduce(
                            ttr_o[:, 0:512],
                            ET1[:, 0:2, :, tbl, :], ET1[:, 2:4, :, tbl, :],
                            1.0, 0.0, op0=ADD, op1=ADD,
                            accum_out=l2a[:, tbl:tbl + 1])
            # drain o1 group 3
            nc.vector.tensor_copy(o1T[:, 1536:2048], oB[1][:])
            for j in (14, 15):
                for q in range(2):
                    nc.tensor.matmul(oC[q][:], x_hi[:, j, :],
                                     E1[:, j, q * 512:(q + 1) * 512],
                                     start=False, stop=(j == NB - 1))

            # ---- final: o1 second half + all epilogues ----
            nc.vector.tensor_reduce(l1[:], l1p[:], axis=AX, op=ADD)
            nc.vector.reciprocal(r1[:], l1[:])
            nc.scalar.copy(o2T[:, 1024:1536], oC[0][:])
            nc.scalar.copy(o2T[:, 1536:2048], oC[1][:])

            erot_tags = [sA[0], sA[1], oC[0], oC[1]]
            erot = [0]
            st1g = [None] * 4
            st2g = [None] * 4

            def epi1_step(j, pin=None):
                # a1 block j: transpose o1T col-block, * r1 * x (DVE stt)
                g = j // 4
                if j % 4 == 0:
                    st1g[g] = stg.tile([P, 4, D], f32, tag="st1",
                                       name=f"st1_{g}")
                bank = erot_tags[erot[0] % 4]
                erot[0] += 1
                tr = nc.tensor.transpose(bank[:, 0:P],
                                         o1T[:, j * P:(j + 1) * P], ident[:])
                if pin is not None:
                    add_dep_helper(tr.ins, pin.ins, sync=False,
                                   reason="epi1 slot")
                nc.vector.scalar_tensor_tensor(st1g[g][:, j % 4, :],
                                               bank[:, 0:P], r1[:, j:j + 1],
                                               x_sb[:, j, :], op0=MUL, op1=MUL)
                if j % 4 == 3:
                    nc.gpsimd.dma_start(out_dv[:, g * 4:(g + 1) * 4, 0:D],
                                        st1g[g][:])

            def epi2_step(j, pin=None, dve=False):
                # a2 block j: transpose o2T col-block, * r2 * y
                g = j // 4
                if j % 4 == 0:
                    st2g[g] = stg.tile([P, 4, D], f32, tag="st2",
                                       name=f"st2_{g}")
                bank = erot_tags[erot[0] % 4]
                erot[0] += 1
                tr = nc.tensor.transpose(bank[:, 0:P],
                                         o2T[:, j * P:(j + 1) * P], ident[:])
                if pin is not None:
                    add_dep_helper(tr.ins, pin.ins, sync=False,
                                   reason="epi2 slot")
                if dve:
                    nc.vector.scalar_tensor_tensor(st2g[g][:, j % 4, :],
                                                   bank[:, 0:P], r2[:, j:j + 1],
                                                   y_sb[:, j, :],
                                                   op0=MUL, op1=MUL)
                else:
                    # ACT scale + GpSimd multiply (keeps DVE free)
                    nc.scalar.activation(sc2[:, j % PB, :], bank[:, 0:P],
                                         Copy, scale=r2[:, j:j + 1])
                    nc.gpsimd.tensor_tensor(st2g[g][:, j % 4, :],
                                            sc2[:, j % PB, :], y_sb[:, j, :],
                                            op=MUL)
                if j % 4 == 3:
                    nc.gpsimd.dma_start(out_dv[:, g * 4:(g + 1) * 4, D:2 * D],
                                        st2g[g][:])

            # o1 second half: 4 groups of 8 (t-blocks 8..15), epis interleaved
            for q in range(4):
                last = None
                for tb in range(PB, NB):
                    last = nc.tensor.matmul(oB[q % 2][:], y_hi[:, tb, :],
                                            o1_mov(tb, q),
                                            start=(tb == PB), stop=(tb == NB - 1))
                    if tb - PB < 4 and q > 0:
                        # previous group's epi1 blocks
                        epi1_step(4 * (q - 1) + (tb - PB), pin=last)
                    elif tb - PB in (4, 5) and q > 0:
                        epi2_step(2 * (q - 1) + (tb - PB - 4), pin=last)
                # drain-add: o1T cols q = first-half + second-half
                nc.vector.tensor_tensor(o1T[:, q * 512:(q + 1) * 512],
                                        o1T[:, q * 512:(q + 1) * 512],
                                        oB[q % 2][:], op=ADD)
            for k in range(4):
                epi1_step(12 + k)
            for k in range(6, 8):
                epi2_step(k)
            # second-half colsums for t-blocks 8..15 (pairs 4..7 of ET1)
            for tbl in range(PB):
                nc.vector.tensor_tensor_reduce(
                    ttr_o[:, 0:512],
                    ET1[:, 4:6, :, tbl, :], ET1[:, 6:8, :, tbl, :],
                    1.0, l2a[:, tbl:tbl + 1], op0=ADD, op1=ADD,
                    accum_out=l2[:, PB + tbl:PB + tbl + 1])
            nc.vector.reciprocal(r2[:, 8:16], l2[:, 8:16])
            for j in range(8, 16):
                epi2_step(j, dve=True)

    nc.compile()
    return nc


def _get_nc():
    global _NC_CACHE
    if _NC_CACHE is None:
        nc = bacc.Bacc("TRN2", target_bir_lowering=False, debug=False,
                       num_devices=B)
        _NC_CACHE = _build_program(nc)
    return _NC_CACHE


def kernel(x, y):
    global LAST_EXEC_NS
    nc = _get_nc()
    x = np.asarray(x, dtype=np.float32)
    y = np.asarray(y, dtype=np.float32)
    in_maps = [
        {"x": np.ascontiguousarray(x[b]), "y": np.ascontiguousarray(y[b])}
        for b in range(B)
    ]
    trace = bool(int(os.environ.get("KERNEL_TRACE", "0")))
    res = run_bass_kernel_spmd(nc, in_maps, list(range(B)), trace=trace)
    LAST_EXEC_NS = res.exec_time_ns
    return np.stack([res.results[b]["out"] for b in range(B)], axis=0)


# revision 10
# speedup vs baseline: 9467.9219x; 9467.9219x over previous
"""BiModal attention kernel for Trainium2 (8 NeuronCores, data-parallel over batch).

Per core (one batch b): x, y: [2048, 128] fp32.
  S = x @ y.T                    (f32r matmuls, [2048, 2048])
  E = exp(S)                     (unshifted; softmax is shift-invariant and
                                  |S| <~ 67 so exp stays in fp32/bf16 range)
  a1 = (E @ y) / rowsum(E) * x
  a2 = (E.T @ x) / colsum(E) * y
  out = concat([a1, a2], -1)     ([2048, 256])

Layout: rows are relabeled r = 16*p + b (p = SBUF partition, b = block index)
so every DRAM transfer is contiguous per partition; applied consistently to s
and t everywhere, so the math is unchanged.

Pipeline (exp on ACT is the 40us spine; everything else hides under it):
  head:    x/y DMA in; PE-transposes build xT/yT (exact f32, HAM warmup)
  panel 0: S(:, :1024) -> exp(+l1 accum) -> paired ET xbar issue (Sync);
           o2 cols :1024 accumulate (lag 2)
  panel 1: S(:, 1024:) -> exp -> ET pairs; o2 cols 1024: accumulate;
           o1 first-half (t 0:1024) accumulation in 2 rotating banks;
           l2 colsums of t-blocks 0..7 (fused tensor_tensor_reduce)
  final:   o1 second half + drain-adds; all gating epilogues, pipelined.
ET is stored pair-interleaved ([tp, pair, i, tb, sp]) so each xbar transpose
moves a contiguous [128, 2048] block; s-chunk order seen by o1 is unchanged.
"""
import sys

sys.path.insert(0, "/opt/trn_rl_repo")

import os
import numpy as np

import concourse.bass as bass
import concourse.mybir as mybir
import concourse.tile as tile
from concourse.tile_rust import add_dep_helper
from concourse import bacc
from concourse.bass_utils import run_bass_kernel_spmd
from concourse.masks import make_identity

f32 = mybir.dt.float32
f32r = mybir.dt.float32r
bf16 = mybir.dt.bfloat16

B = 8
S = 2048
D = 128
P = 128
NB = S // P          # 16 blocks
NP = 2               # panels
PW = S // NP         # panel width (1024)
PB = PW // P         # t-blocks per panel (8)
KP = PB              # transpose pairs per panel (8)

_NC_CACHE = None
LAST_EXEC_NS = None


def _build_program(nc):
    x_d = nc.dram_tensor("x", [S, D], f32, kind="ExternalInput").ap()
    y_d = nc.dram_tensor("y", [S, D], f32, kind="ExternalInput").ap()
    out_d = nc.dram_tensor("out", [S, 2 * D], f32, kind="ExternalOutput").ap()

    x_dv = x_d.rearrange("(p b) d -> p b d", p=P)      # [128, 16, 128]
    y_dv = y_d.rearrange("(p b) d -> p b d", p=P)
    out_dv = out_d.rearrange("(p b) c -> p b c", p=P)  # [128, 16, 256]

    Exp = mybir.ActivationFunctionType.Exp
    Copy = mybir.ActivationFunctionType.Copy
    MUL = mybir.AluOpType.mult
    ADD = mybir.AluOpType.add
    AX = mybir.AxisListType.X

    with tile.TileContext(nc) as tc:
        with (
            tc.tile_pool(name="sb", bufs=1) as sb,
            tc.tile_pool(name="stg", bufs=2) as stg,
            tc.tile_pool(name="ps", bufs=1, space="PSUM") as ps,
        ):
            # ---- persistent SBUF tensors ----
            x_sb = sb.tile([P, NB, D], f32, tag="x_sb")
            y_sb = sb.tile([P, NB, D], f32, tag="y_sb")
            xT = sb.tile([P, NB, P], f32r, tag="xT")       # [d, sb, sp]
            yT = sb.tile([P, NB, P], f32r, tag="yT")       # [d, tb, tp]
            x_hi = sb.tile([P, NB, D], bf16, tag="x_hi")
            y_hi = sb.tile([P, NB, D], bf16, tag="y_hi")
            E0 = sb.tile([P, NB, PW], bf16, tag="E0")      # [sp, sb, t<1024]
            E1 = sb.tile([P, NB, PW], bf16, tag="E1")      # [sp, sb, t-1024]
            ET = sb.tile([P, NB, S], bf16, tag="ET")       # [tp, tb, s]
            o1T = sb.tile([P, S], f32, tag="o1T")          # [d, s]
            o2T = sb.tile([P, S], f32, tag="o2T")          # [d, t]
            ident = sb.tile([P, P], f32, tag="ident")
            l1p = sb.tile([P, NB, 2], f32, tag="l1p")
            l1 = sb.tile([P, NB], f32, tag="l1")
            r1 = sb.tile([P, NB], f32, tag="r1")
            l2a = sb.tile([P, PB], f32, tag="l2a")
            l2 = sb.tile([P, NB], f32, tag="l2")
            r2 = sb.tile([P, NB], f32, tag="r2")
            ttr_o = sb.tile([P, PW], bf16, tag="ttr_o")    # ttr scratch
            sc2 = sb.tile([P, PB, D], f32, tag="sc2")      # epi2 ACT scale out

            # PSUM: A0/A1 [P,1024] = S rotation -> final epi rotation
            #       B0/B1 [P,512]  = o2 q0/q1 -> o1 group rotation
            #       C0/C1 [P,512]  = transpose staging -> o2 q2/q3 -> epi rot
            sA = [ps.tile([P, PW], f32, tag="A0", name="sA0"),
                  ps.tile([P, PW], f32, tag="A1", name="sA1")]
            oB = [ps.tile([P, 512], f32, tag="B0", name="oB0"),
                  ps.tile([P, 512], f32, tag="B1", name="oB1")]
            oC = [ps.tile([P, 512], f32, tag="C0", name="oC0"),
                  ps.tile([P, 512], f32, tag="C1", name="oC1")]

            make_identity(nc, ident[:])

            # ---- input DMA (Sync queue; ET pair issues follow) ----
            nc.sync.dma_start(y_sb[:, 0:4], y_dv[:, 0:4])
            nc.sync.dma_start(x_sb[:, 0:4], x_dv[:, 0:4])
            nc.sync.dma_start(y_sb[:, 4:8], y_dv[:, 4:8])
            nc.sync.dma_start(x_sb[:, 4:8], x_dv[:, 4:8])
            nc.sync.dma_start(y_sb[:, 8:16], y_dv[:, 8:16])
            nc.sync.dma_start(x_sb[:, 8:16], x_dv[:, 8:16])

            # ---- head: PE transposes (warmup + exact f32 xT/yT) ----
            ntr = [0]

            def block_T(which, b, eng):
                v_sb, vT = (x_sb, xT) if which == "x" else (y_sb, yT)
                k = ntr[0] % 2
                ntr[0] += 1
                tr = nc.tensor.transpose(oC[k][:, 0:P], v_sb[:, b, :], ident[:])
                nc.vector.tensor_copy(vT[:, b, :], oC[k][:, 0:P])
                return tr

            for which, b in [("y", 0), ("y", 1), ("y", 2), ("y", 3), ("x", 0)]:
                block_T(which, b, nc.scalar)
            for which, b in [("y", 4), ("y", 5), ("y", 6), ("y", 7),
                             ("x", 1), ("x", 2), ("x", 3)]:
                block_T(which, b, nc.vector)
            rest_list = ([("x", b) for b in range(4, 16)]
                         + [("y", b) for b in range(8, 16)])

            nc.vector.tensor_copy(x_hi[:, 0:8], x_sb[:, 0:8])

            yT_f = yT[:].rearrange("p b d -> p (b d)")
            E_t = [E0, E1]

            def et_one(ct, i):
                # transpose E row-block i (panel ct): baseline-identical call
                nc.sync.dma_start_transpose(
                    ET[:, ct * PB:(ct + 1) * PB, i * P:(i + 1) * P],
                    E_t[ct][:, i, :])

            def o1_mov(tb, q):
                return ET[:, tb, q * 512:(q + 1) * 512]

            # ---- panel 0 ----
            for i in range(NB):
                slot = sA[i % 2][:]
                nc.tensor.matmul(slot[:, 0:512], xT[:, i, :],
                                 yT_f[:, 0:512], start=True, stop=True)
                sm = nc.tensor.matmul(slot[:, 512:1024], xT[:, i, :],
                                      yT_f[:, 512:1024], start=True, stop=True)
                if i >= 2:
                    for _ in range(2):
                        if rest_list:
                            tr = block_T(*rest_list.pop(0), nc.vector)
                            add_dep_helper(tr.ins, sm.ins, sync=False,
                                           reason="keep transpose at slot")
                    j = i - 2
                    for q in range(2):
                        om = nc.tensor.matmul(oB[q][:], x_hi[:, j, :],
                                              E0[:, j, q * 512:(q + 1) * 512],
                                              start=(j == 0), stop=(j == NB - 1))
                        if q == 0:
                            add_dep_helper(om.ins, sm.ins, sync=False,
                                           reason="keep o2 at slot")
                nc.scalar.activation(E0[:, i, :], slot, Exp,
                                     accum_out=l1p[:, i, 0:1])
                et_one(0, i)
                if i == 4:
                    nc.vector.tensor_copy(x_hi[:, 8:16], x_sb[:, 8:16])
                if i == 6:
                    nc.vector.tensor_copy(y_hi[:, 0:8], y_sb[:, 0:8])
                if i == 8:
                    nc.vector.tensor_copy(y_hi[:, 8:16], y_sb[:, 8:16])
            for j in (14, 15):
                for q in range(2):
                    nc.tensor.matmul(oB[q][:], x_hi[:, j, :],
                                     E0[:, j, q * 512:(q + 1) * 512],
                                     start=False, stop=(j == NB - 1))

            # ---- panel 1 ----
            for i in range(NB):
                slot = sA[i % 2][:]
                nc.tensor.matmul(slot[:, 0:512], xT[:, i, :],
                                 yT_f[:, 1024:1536], start=True, stop=True)
                sm = nc.tensor.matmul(slot[:, 512:1024], xT[:, i, :],
                                      yT_f[:, 1536:2048], start=True, stop=True)
                if i == 0:
                    # drain o2 cols 0:1024; B banks go to o1 groups
                    nc.vector.tensor_copy(o2T[:, 0:512], oB[0][:])
                    nc.vector.tensor_copy(o2T[:, 512:1024], oB[1][:])
                # o1 first half: group q = i//4 accumulates t-blocks 0..7
                q = i // 4
                for z in range(2):
                    tb = 2 * (i % 4) + z
                    o1m = nc.tensor.matmul(oB[q % 2][:], y_hi[:, tb, :],
                                           o1_mov(tb, q),
                                           start=(tb == 0), stop=(tb == PB - 1))
                    if z == 0:
                        add_dep_helper(o1m.ins, sm.ins, sync=False,
                                       reason="keep o1 at slot")
                if i % 4 == 0 and i >= 4:
                    # previous o1 group finished last iter -> drain it
                    pq = q - 1
                    nc.vector.tensor_copy(o1T[:, pq * 512:(pq + 1) * 512],
                                          oB[pq % 2][:])
                if i >= 2:
                    j = i - 2
                    for qq in range(2):
                        om = nc.tensor.matmul(oC[qq][:], x_hi[:, j, :],
                                              E1[:, j, qq * 512:(qq + 1) * 512],
                                              start=(j == 0), stop=(j == NB - 1))
                        if qq == 0:
                            add_dep_helper(om.ins, sm.ins, sync=False,
                                           reason="keep o2 at slot")
                nc.scalar.activation(E1[:, i, :], slot, Exp,
                                     accum_out=l1p[:, i, 1:2])
                et_one(1, i)
                # l2 for t-blocks 0..7 (full colsum, one fused op each)
                if 1 <= i < 9:
                    tb = i - 1
                    nc.vector.tensor_reduce(l2[:, tb:tb + 1], ET[:, tb, :],
                                            axis=AX, op=ADD)
                if i == 9:
                    nc.vector.reciprocal(r2[:, 0:8], l2[:, 0:8])
                # first-half colsums for t-blocks 8..15 (pairs 0..3 of ET1)
                if i >= 12:
                    for z in range(2):
                        tbl = 2 * (i - 12) + z
                        tbg = PB + tbl
                        nc.vector.tensor_reduce(l2a[:, tbl:tbl + 1],
                                                ET[:, tbg, 0:1024],
                                                axis=AX, op=ADD)
            # drain o1 group 3
            nc.vector.tensor_copy(o1T[:, 1536:2048], oB[1][:])
            for j in (14, 15):
                for q in range(2):
                    nc.tensor.matmul(oC[q][:], x_hi[:, j, :],
                                     E1[:, j, q * 512:(q + 1) * 512],
                                     start=False, stop=(j == NB - 1))

            # ---- final: o1 second half + all epilogues ----
            nc.vector.tensor_reduce(l1[:], l1p[:], axis=AX, op=ADD)
            nc.vector.reciprocal(r1[:], l1[:])
            nc.scalar.copy(o2T[:, 1024:1536], oC[0][:])
            nc.scalar.copy(o2T[:, 1536:2048], oC[1][:])

            erot_tags = [sA[0], sA[1], oC[0], oC[1]]
            erot = [0]
            st1g = [None] * 4
            st2g = [None] * 4

            def epi1_step(j, pin=None):
                # a1 block j: transpose o1T col-block, * r1 * x (DVE stt)
                g = j // 4
                if j % 4 == 0:
                    st1g[g] = stg.tile([P, 4, D], f32, tag="st1",
                                       name=f"st1_{g}")
                bank = erot_tags[erot[0] % 4]
                erot[0] += 1
                tr = nc.tensor.transpose(bank[:, 0:P],
                                         o1T[:, j * P:(j + 1) * P], ident[:])
                if pin is not None:
                    add_dep_helper(tr.ins, pin.ins, sync=False,
                                   reason="epi1 slot")
                nc.vector.scalar_tensor_tensor(st1g[g][:, j % 4, :],
                                               bank[:, 0:P], r1[:, j:j + 1],
                                               x_sb[:, j, :], op0=MUL, op1=MUL)
                if j % 4 == 3:
                    nc.gpsimd.dma_start(out_dv[:, g * 4:(g + 1) * 4, 0:D],
                                        st1g[g][:])

            def epi2_step(j, pin=None, dve=False):
                # a2 block j: transpose o2T col-block, * r2 * y
                g = j // 4
                if j % 4 == 0:
                    st2g[g] = stg.tile([P, 4, D], f32, tag="st2",
                                       name=f"st2_{g}")
                bank = erot_tags[erot[0] % 4]
                erot[0] += 1
                tr = nc.tensor.transpose(bank[:, 0:P],
                                         o2T[:, j * P:(j + 1) * P], ident[:])
                if pin is not None:
                    add_dep_helper(tr.ins, pin.ins, sync=False,
                                   reason="epi2 slot")
                nc.vector.scalar_tensor_tensor(st2g[g][:, j % 4, :],
                                               bank[:, 0:P], r2[:, j:j + 1],
                                               y_sb[:, j, :],
                                               op0=MUL, op1=MUL)
                if j % 4 == 3:
                    nc.gpsimd.dma_start(out_dv[:, g * 4:(g + 1) * 4, D:2 * D],
                                        st2g[g][:])

            # o1 second half: 4 groups of 8 (t-blocks 8..15), epis interleaved
            for q in range(4):
                last = None
                for tb in range(PB, NB):
                    last = nc.tensor.matmul(oB[q % 2][:], y_hi[:, tb, :],
                                            o1_mov(tb, q),
                                            start=(tb == PB), stop=(tb == NB - 1))
                    if tb - PB < 4 and q > 0:
                        # previous group's epi1 blocks
                        epi1_step(4 * (q - 1) + (tb - PB), pin=last)
                    elif tb - PB in (4, 5) and q > 0:
                        epi2_step(2 * (q - 1) + (tb - PB - 4), pin=last)
                # drain-add: o1T cols q = first-half + second-half
                nc.vector.tensor_tensor(o1T[:, q * 512:(q + 1) * 512],
                                        o1T[:, q * 512:(q + 1) * 512],
                                        oB[q % 2][:], op=ADD)
            for k in range(4):
                epi1_step(12 + k)
            for k in range(6, 8):
                epi2_step(k)
            # second-half colsums for t-blocks 8..15 (pairs 4..7 of ET1)
            for tbl in range(PB):
                tbg = PB + tbl
                nc.vector.tensor_reduce(l2[:, tbg:tbg + 1],
                                        ET[:, tbg, 1024:2048],
                                        axis=AX, op=ADD)
                nc.vector.tensor_tensor(l2[:, tbg:tbg + 1], l2[:, tbg:tbg + 1],
                                        l2a[:, tbl:tbl + 1], op=ADD)
            nc.vector.reciprocal(r2[:, 8:16], l2[:, 8:16])
            for j in range(8, 16):
                epi2_step(j, dve=True)

    nc.compile()
    return nc


def _get_nc():
    global _NC_CACHE
    if _NC_CACHE is None:
        nc = bacc.Bacc("TRN2", target_bir_lowering=False, debug=False,
                       num_devices=B)
        _NC_CACHE = _build_program(nc)
    return _NC_CACHE


def kernel(x, y):
    global LAST_EXEC_NS
    nc = _get_nc()
    x = np.asarray(x, dtype=np.float32)
    y = np.asarray(y, dtype=np.float32)
    in_maps = [
        {"x": np.ascontiguousarray(x[b]), "y": np.ascontiguousarray(y[b])}
        for b in range(B)
    ]
    trace = bool(int(os.environ.get("KERNEL_TRACE", "0")))
    res = run_bass_kernel_spmd(nc, in_maps, list(range(B)), trace=trace)
    LAST_EXEC_NS = res.exec_time_ns
    return np.stack([res.results[b]["out"] for b in range(B)], axis=0)


# revision 13
# speedup vs baseline: 10385.7588x; 1.0969x over previous
"""BiModal attention kernel for Trainium2 (8 NeuronCores, data-parallel over batch).

Per core (one batch b): x, y: [2048, 128] fp32.
  S = x @ y.T                    (f32r matmuls, [2048, 2048])
  E = exp(S)                     (unshifted; softmax is shift-invariant and
                                  |S| <~ 67 so exp stays in fp32/bf16 range)
  a1 = (E @ y) / rowsum(E) * x
  a2 = (E.T @ x) / colsum(E) * y
  out = concat([a1, a2], -1)     ([2048, 256])

Layout: rows are relabeled r = 16*p + b (p = SBUF partition, b = block index)
so every DRAM transfer is contiguous per partition; applied consistently to s
and t everywhere, so the math is unchanged.

Pipeline (exp on ACT is the 40us spine; everything else hides under it):
  head:    x/y DMA in; PE-transposes build xT/yT (exact f32, HAM warmup)
  panel 0: S(:, :1024) -> exp(+l1 accum) -> paired ET xbar issue (Sync);
           o2 cols :1024 accumulate (lag 2)
  panel 1: S(:, 1024:) -> exp -> ET pairs; o2 cols 1024: accumulate;
           o1 first-half (t 0:1024) accumulation in 2 rotating banks;
           l2 colsums of t-blocks 0..7 (fused tensor_tensor_reduce)
  final:   o1 second half + drain-adds; all gating epilogues, pipelined.
ET is stored pair-interleaved ([tp, pair, i, tb, sp]) so each xbar transpose
moves a contiguous [128, 2048] block; s-chunk order seen by o1 is unchanged.
"""
import sys

sys.path.insert(0, "/opt/trn_rl_repo")

import os
import numpy as np

import concourse.bass as bass
import concourse.mybir as mybir
import concourse.tile as tile
from concourse.tile_rust import add_dep_helper
from concourse import bacc
from concourse.bass_utils import run_bass_kernel_spmd
from concourse.masks import make_identity

f32 = mybir.dt.float32
f32r = mybir.dt.float32r
bf16 = mybir.dt.bfloat16

B = 8
S = 2048
D = 128
P = 128
NB = S // P          # 16 blocks
NP = 2               # panels
PW = S // NP         # panel width (1024)
PB = PW // P         # t-blocks per panel (8)
KP = PB              # transpose pairs per panel (8)

_NC_CACHE = None
LAST_EXEC_NS = None


def _build_program(nc):
    x_d = nc.dram_tensor("x", [S, D], f32, kind="ExternalInput").ap()
    y_d = nc.dram_tensor("y", [S, D], f32, kind="ExternalInput").ap()
    out_d = nc.dram_tensor("out", [S, 2 * D], f32, kind="ExternalOutput").ap()

    x_dv = x_d.rearrange("(p b) d -> p b d", p=P)      # [128, 16, 128]
    y_dv = y_d.rearrange("(p b) d -> p b d", p=P)
    out_dv = out_d.rearrange("(p b) c -> p b c", p=P)  # [128, 16, 256]

    Exp = mybir.ActivationFunctionType.Exp
    Copy = mybir.ActivationFunctionType.Copy
    MUL = mybir.AluOpType.mult
    ADD = mybir.AluOpType.add
    AX = mybir.AxisListType.X

    with tile.TileContext(nc) as tc:
        with (
            tc.tile_pool(name="sb", bufs=1) as sb,
            tc.tile_pool(name="stg", bufs=2) as stg,
            tc.tile_pool(name="ps", bufs=1, space="PSUM") as ps,
        ):
            # ---- persistent SBUF tensors ----
            x_sb = sb.tile([P, NB, D], f32, tag="x_sb")
            y_sb = sb.tile([P, NB, D], f32, tag="y_sb")
            xT = sb.tile([P, NB, P], f32r, tag="xT")       # [d, sb, sp]
            yT = sb.tile([P, NB, P], f32r, tag="yT")       # [d, tb, tp]
            x_hi = sb.tile([P, NB, D], bf16, tag="x_hi")
            y_hi = sb.tile([P, NB, D], bf16, tag="y_hi")
            E0 = sb.tile([P, NB, PW], bf16, tag="E0")      # [sp, sb, t<1024]
            E1 = sb.tile([P, NB, PW], bf16, tag="E1")      # [sp, sb, t-1024]
            ET = sb.tile([P, NB, S], bf16, tag="ET")       # [tp, tb, s]
            o1T = sb.tile([P, S], f32, tag="o1T")          # [d, s]
            o2T = sb.tile([P, S], f32, tag="o2T")          # [d, t]
            ident = sb.tile([P, P], f32, tag="ident")
            l1p = sb.tile([P, NB, 2], f32, tag="l1p")
            l1 = sb.tile([P, NB], f32, tag="l1")
            r1 = sb.tile([P, NB], f32, tag="r1")
            l2a = sb.tile([P, PB], f32, tag="l2a")
            l2 = sb.tile([P, NB], f32, tag="l2")
            r2 = sb.tile([P, NB], f32, tag="r2")
            scr = sb.tile([P, S], bf16, tag="scr")         # ACT accum scratch
            sc2 = sb.tile([P, PB, D], f32, tag="sc2")      # epi2 ACT scale out

            # PSUM: A0/A1 [P,1024] = S rotation -> final epi rotation
            #       B0/B1 [P,512]  = o2 q0/q1 -> o1 group rotation
            #       C0/C1 [P,512]  = transpose staging -> o2 q2/q3 -> epi rot
            sA = [ps.tile([P, PW], f32, tag="A0", name="sA0"),
                  ps.tile([P, PW], f32, tag="A1", name="sA1")]
            oB = [ps.tile([P, 512], f32, tag="B0", name="oB0"),
                  ps.tile([P, 512], f32, tag="B1", name="oB1")]
            oC = [ps.tile([P, 512], f32, tag="C0", name="oC0"),
                  ps.tile([P, 512], f32, tag="C1", name="oC1")]

            make_identity(nc, ident[:])

            # ---- input DMA (Sync queue; ET pair issues follow) ----
            nc.sync.dma_start(y_sb[:, 0:4], y_dv[:, 0:4])
            nc.sync.dma_start(x_sb[:, 0:4], x_dv[:, 0:4])
            nc.sync.dma_start(y_sb[:, 4:8], y_dv[:, 4:8])
            nc.sync.dma_start(x_sb[:, 4:8], x_dv[:, 4:8])
            nc.sync.dma_start(y_sb[:, 8:16], y_dv[:, 8:16])
            nc.sync.dma_start(x_sb[:, 8:16], x_dv[:, 8:16])

            # ---- head: PE transposes (warmup + exact f32 xT/yT) ----
            ntr = [0]

            def block_T(which, b, eng):
                v_sb, vT = (x_sb, xT) if which == "x" else (y_sb, yT)
                k = ntr[0] % 2
                ntr[0] += 1
                tr = nc.tensor.transpose(oC[k][:, 0:P], v_sb[:, b, :], ident[:])
                nc.vector.tensor_copy(vT[:, b, :], oC[k][:, 0:P])
                return tr

            for which, b in [("y", 0), ("y", 1), ("y", 2), ("y", 3), ("x", 0)]:
                block_T(which, b, nc.scalar)
            for which, b in [("y", 4), ("y", 5), ("y", 6), ("y", 7),
                             ("x", 1), ("x", 2), ("x", 3)]:
                block_T(which, b, nc.vector)
            rest_list = ([("x", b) for b in range(4, 16)]
                         + [("y", b) for b in range(8, 16)])

            nc.vector.tensor_copy(x_hi[:, 0:8], x_sb[:, 0:8])

            yT_f = yT[:].rearrange("p b d -> p (b d)")
            E_t = [E0, E1]

            def et_one(ct, i):
                # transpose E row-block i (panel ct): baseline-identical call
                nc.sync.dma_start_transpose(
                    ET[:, ct * PB:(ct + 1) * PB, i * P:(i + 1) * P],
                    E_t[ct][:, i, :])

            def o1_mov(tb, q):
                return ET[:, tb, q * 512:(q + 1) * 512]

            # ---- panel 0 ----
            for i in range(NB):
                slot = sA[i % 2][:]
                nc.tensor.matmul(slot[:, 0:512], xT[:, i, :],
                                 yT_f[:, 0:512], start=True, stop=True)
                sm = nc.tensor.matmul(slot[:, 512:1024], xT[:, i, :],
                                      yT_f[:, 512:1024], start=True, stop=True)
                if i >= 2:
                    for _ in range(2):
                        if rest_list:
                            tr = block_T(*rest_list.pop(0), nc.vector)
                            add_dep_helper(tr.ins, sm.ins, sync=False,
                                           reason="keep transpose at slot")
                    j = i - 2
                    for q in range(2):
                        om = nc.tensor.matmul(oB[q][:], x_hi[:, j, :],
                                              E0[:, j, q * 512:(q + 1) * 512],
                                              start=(j == 0), stop=(j == NB - 1))
                        if q == 0:
                            add_dep_helper(om.ins, sm.ins, sync=False,
                                           reason="keep o2 at slot")
                nc.scalar.activation(E0[:, i, :], slot, Exp,
                                     accum_out=l1p[:, i, 0:1])
                et_one(0, i)
                if i == 4:
                    nc.vector.tensor_copy(x_hi[:, 8:16], x_sb[:, 8:16])
                if i == 6:
                    nc.vector.tensor_copy(y_hi[:, 0:8], y_sb[:, 0:8])
                if i == 8:
                    nc.vector.tensor_copy(y_hi[:, 8:16], y_sb[:, 8:16])
            for j in (14, 15):
                for q in range(2):
                    nc.tensor.matmul(oB[q][:], x_hi[:, j, :],
                                     E0[:, j, q * 512:(q + 1) * 512],
                                     start=False, stop=(j == NB - 1))

            # ---- panel 1 ----
            for i in range(NB):
                slot = sA[i % 2][:]
                nc.tensor.matmul(slot[:, 0:512], xT[:, i, :],
                                 yT_f[:, 1024:1536], start=True, stop=True)
                sm = nc.tensor.matmul(slot[:, 512:1024], xT[:, i, :],
                                      yT_f[:, 1536:2048], start=True, stop=True)
                if i == 0:
                    # drain o2 cols 0:1024; B banks go to o1 groups
                    nc.vector.tensor_copy(o2T[:, 0:512], oB[0][:])
                    nc.vector.tensor_copy(o2T[:, 512:1024], oB[1][:])
                # o1 first half: group q = i//4 accumulates t-blocks 0..7
                q = i // 4
                for z in range(2):
                    tb = 2 * (i % 4) + z
                    o1m = nc.tensor.matmul(oB[q % 2][:], y_hi[:, tb, :],
                                           o1_mov(tb, q),
                                           start=(tb == 0), stop=(tb == PB - 1))
                    if z == 0:
                        add_dep_helper(o1m.ins, sm.ins, sync=False,
                                       reason="keep o1 at slot")
                if i % 4 == 0 and i >= 4:
                    # previous o1 group finished last iter -> drain it
                    pq = q - 1
                    nc.vector.tensor_copy(o1T[:, pq * 512:(pq + 1) * 512],
                                          oB[pq % 2][:])
                if i >= 2:
                    j = i - 2
                    for qq in range(2):
                        om = nc.tensor.matmul(oC[qq][:], x_hi[:, j, :],
                                              E1[:, j, qq * 512:(qq + 1) * 512],
                                              start=(j == 0), stop=(j == NB - 1))
                        if qq == 0:
                            add_dep_helper(om.ins, sm.ins, sync=False,
                                           reason="keep o2 at slot")
                nc.scalar.activation(E1[:, i, :], slot, Exp,
                                     accum_out=l1p[:, i, 1:2])
                et_one(1, i)
                # l2 for t-blocks 0..7 (full colsum, one fused op each)
                if i % 2 == 1:
                    tb = (i - 1) // 2
                    nc.vector.tensor_reduce(l2[:, tb:tb + 1], ET[:, tb, :],
                                            axis=AX, op=ADD)
            # drain o1 group 3
            nc.vector.tensor_copy(o1T[:, 1536:2048], oB[1][:])
            for j in (14, 15):
                for q in range(2):
                    nc.tensor.matmul(oC[q][:], x_hi[:, j, :],
                                     E1[:, j, q * 512:(q + 1) * 512],
                                     start=False, stop=(j == NB - 1))

            # ---- final: o1 second half + all epilogues ----
            nc.vector.tensor_reduce(l1[:], l1p[:], axis=AX, op=ADD)
            nc.vector.reciprocal(r1[:], l1[:])
            nc.vector.reciprocal(r2[:, 0:8], l2[:, 0:8])
            nc.scalar.copy(o2T[:, 1024:1536], oC[0][:])
            nc.scalar.copy(o2T[:, 1536:2048], oC[1][:])

            erot_tags = [sA[0], sA[1], oC[0], oC[1]]
            erot = [0]
            st1g = [None] * 4
            st2g = [None] * 4

            def epi1_step(j, pin=None):
                # a1 block j: transpose o1T col-block, * r1 * x (DVE stt)
                g = j // 4
                if j % 4 == 0:
                    st1g[g] = stg.tile([P, 4, D], f32, tag="st1",
                                       name=f"st1_{g}")
                bank = erot_tags[erot[0] % 4]
                erot[0] += 1
                tr = nc.tensor.transpose(bank[:, 0:P],
                                         o1T[:, j * P:(j + 1) * P], ident[:])
                if pin is not None:
                    add_dep_helper(tr.ins, pin.ins, sync=False,
                                   reason="epi1 slot")
                nc.vector.scalar_tensor_tensor(st1g[g][:, j % 4, :],
                                               bank[:, 0:P], r1[:, j:j + 1],
                                               x_sb[:, j, :], op0=MUL, op1=MUL)
                if j % 4 == 3:
                    nc.gpsimd.dma_start(out_dv[:, g * 4:(g + 1) * 4, 0:D],
                                        st1g[g][:])

            def epi2_step(j, pin=None, dve=False):
                # a2 block j: transpose o2T col-block, * r2 * y
                g = j // 4
                if j % 4 == 0:
                    st2g[g] = stg.tile([P, 4, D], f32, tag="st2",
                                       name=f"st2_{g}")
                bank = erot_tags[erot[0] % 4]
                erot[0] += 1
                tr = nc.tensor.transpose(bank[:, 0:P],
                                         o2T[:, j * P:(j + 1) * P], ident[:])
                if pin is not None:
                    add_dep_helper(tr.ins, pin.ins, sync=False,
                                   reason="epi2 slot")
                nc.scalar.activation(sc2[:, j % PB, :], bank[:, 0:P],
                                     Copy, scale=r2[:, j:j + 1])
                nc.gpsimd.tensor_tensor(st2g[g][:, j % 4, :],
                                        sc2[:, j % PB, :], y_sb[:, j, :],
                                        op=MUL)
                if j % 4 == 3:
                    nc.gpsimd.dma_start(out_dv[:, g * 4:(g + 1) * 4, D:2 * D],
                                        st2g[g][:])

            # o1 second half: 4 groups of 8 (t-blocks 8..15), epis interleaved
            for q in range(4):
                last = None
                for tb in range(PB, NB):
                    last = nc.tensor.matmul(oB[q % 2][:], y_hi[:, tb, :],
                                            o1_mov(tb, q),
                                            start=(tb == PB), stop=(tb == NB - 1))
                    if tb - PB < 4 and q > 0:
                        # previous group's epi1 blocks
                        epi1_step(4 * (q - 1) + (tb - PB), pin=last)
                    elif tb - PB in (4, 5) and q > 0:
                        epi2_step(2 * (q - 1) + (tb - PB - 4), pin=last)
                # drain-add: o1T cols q = first-half + second-half
                nc.vector.tensor_tensor(o1T[:, q * 512:(q + 1) * 512],
                                        o1T[:, q * 512:(q + 1) * 512],
                                        oB[q % 2][:], op=ADD)
            for k in range(4):
                epi1_step(12 + k)
            for k in range(6, 8):
                epi2_step(k)
            # colsums for t-blocks 8..15: split across ACT and DVE
            for tbg in range(PB, PB + 4):
                nc.scalar.activation(scr[:], ET[:, tbg, :], Copy,
                                     accum_out=l2[:, tbg:tbg + 1])
            for tbg in range(PB + 4, NB):
                nc.vector.tensor_reduce(l2[:, tbg:tbg + 1], ET[:, tbg, :],
                                        axis=AX, op=ADD)
            nc.vector.reciprocal(r2[:, 8:16], l2[:, 8:16])
            for j in range(8, 16):
                epi2_step(j)

    nc.compile()
    return nc


def _get_nc():
    global _NC_CACHE
    if _NC_CACHE is None:
        nc = bacc.Bacc("TRN2", target_bir_lowering=False, debug=False,
                       num_devices=B)
        _NC_CACHE = _build_program(nc)
    return _NC_CACHE


def kernel(x, y):
    global LAST_EXEC_NS
    nc = _get_nc()
    x = np.asarray(x, dtype=np.float32)
    y = np.asarray(y, dtype=np.float32)
    in_maps = [
        {"x": np.ascontiguousarray(x[b]), "y": np.ascontiguousarray(y[b])}
        for b in range(B)
    ]
    trace = bool(int(os.environ.get("KERNEL_TRACE", "0")))
    res = run_bass_kernel_spmd(nc, in_maps, list(range(B)), trace=trace)
    LAST_EXEC_NS = res.exec_time_ns
    return np.stack([res.results[b]["out"] for b in range(B)], axis=0)


# revision 14
# speedup vs baseline: 10853.8962x; 1.0451x over previous
"""BiModal attention kernel for Trainium2 (8 NeuronCores, data-parallel over batch).

Per core (one batch b): x, y: [2048, 128] fp32.
  S = x @ y.T                    (f32r matmuls, [2048, 2048])
  E = exp(S)                     (unshifted; softmax is shift-invariant and
                                  |S| <~ 67 so exp stays in fp32/bf16 range)
  a1 = (E @ y) / rowsum(E) * x
  a2 = (E.T @ x) / colsum(E) * y
  out = concat([a1, a2], -1)     ([2048, 256])

Layout: rows are relabeled r = 16*p + b (p = SBUF partition, b = block index)
so every DRAM transfer is contiguous per partition; applied consistently to s
and t everywhere, so the math is unchanged.

Pipeline (exp on ACT is the 40us spine; everything else hides under it):
  head:    x/y DMA in; PE-transposes build xT/yT (exact f32, HAM warmup)
  panel 0: S(:, :1024) -> exp(+l1 accum) -> paired ET xbar issue (Sync);
           o2 cols :1024 accumulate (lag 2)
  panel 1: S(:, 1024:) -> exp -> ET pairs; o2 cols 1024: accumulate;
           o1 first-half (t 0:1024) accumulation in 2 rotating banks;
           l2 colsums of t-blocks 0..7 (fused tensor_tensor_reduce)
  final:   o1 second half + drain-adds; all gating epilogues, pipelined.
ET is stored pair-interleaved ([tp, pair, i, tb, sp]) so each xbar transpose
moves a contiguous [128, 2048] block; s-chunk order seen by o1 is unchanged.
"""
import sys

sys.path.insert(0, "/opt/trn_rl_repo")

import os
import numpy as np

import concourse.bass as bass
import concourse.mybir as mybir
import concourse.tile as tile
from concourse.tile_rust import add_dep_helper
from concourse import bacc
from concourse.bass_utils import run_bass_kernel_spmd
from concourse.masks import make_identity

f32 = mybir.dt.float32
f32r = mybir.dt.float32r
bf16 = mybir.dt.bfloat16

B = 8
S = 2048
D = 128
P = 128
NB = S // P          # 16 blocks
NP = 2               # panels
PW = S // NP         # panel width (1024)
PB = PW // P         # t-blocks per panel (8)
KP = PB              # transpose pairs per panel (8)

_NC_CACHE = None
LAST_EXEC_NS = None


def _build_program(nc):
    x_d = nc.dram_tensor("x", [S, D], f32, kind="ExternalInput").ap()
    y_d = nc.dram_tensor("y", [S, D], f32, kind="ExternalInput").ap()
    out_d = nc.dram_tensor("out", [S, 2 * D], f32, kind="ExternalOutput").ap()

    x_dv = x_d.rearrange("(p b) d -> p b d", p=P)      # [128, 16, 128]
    y_dv = y_d.rearrange("(p b) d -> p b d", p=P)
    out_dv = out_d.rearrange("(p b) c -> p b c", p=P)  # [128, 16, 256]

    Exp = mybir.ActivationFunctionType.Exp
    Copy = mybir.ActivationFunctionType.Copy
    MUL = mybir.AluOpType.mult
    ADD = mybir.AluOpType.add
    AX = mybir.AxisListType.X

    with tile.TileContext(nc) as tc:
        with (
            tc.tile_pool(name="sb", bufs=1) as sb,
            tc.tile_pool(name="stg", bufs=2) as stg,
            tc.tile_pool(name="ps", bufs=1, space="PSUM") as ps,
        ):
            # ---- persistent SBUF tensors ----
            x_sb = sb.tile([P, NB, D], f32, tag="x_sb")
            y_sb = sb.tile([P, NB, D], f32, tag="y_sb")
            xT = sb.tile([P, NB, P], f32r, tag="xT")       # [d, sb, sp]
            yT = sb.tile([P, NB, P], f32r, tag="yT")       # [d, tb, tp]
            x_hi = sb.tile([P, NB, D], bf16, tag="x_hi")
            y_hi = sb.tile([P, NB, D], bf16, tag="y_hi")
            E0 = sb.tile([P, NB, PW], bf16, tag="E0")      # [sp, sb, t<1024]
            E1 = sb.tile([P, NB, PW], bf16, tag="E1")      # [sp, sb, t-1024]
            ET = sb.tile([P, NB, S], bf16, tag="ET")       # [tp, tb, s]
            o1T = sb.tile([P, S], f32, tag="o1T")          # [d, s]
            o2T = sb.tile([P, S], f32, tag="o2T")          # [d, t]
            ident = sb.tile([P, P], f32, tag="ident")
            l1p = sb.tile([P, NB, 2], f32, tag="l1p")
            l1 = sb.tile([P, NB], f32, tag="l1")
            r1 = sb.tile([P, NB], f32, tag="r1")
            l2a = sb.tile([P, PB], f32, tag="l2a")
            l2 = sb.tile([P, NB], f32, tag="l2")
            r2 = sb.tile([P, NB], f32, tag="r2")
            scr = sb.tile([P, S], bf16, tag="scr")         # ACT accum scratch
            sc2 = sb.tile([P, PB, D], f32, tag="sc2")      # epi2 ACT scale out

            # PSUM: A0/A1 [P,1024] = S rotation -> final epi rotation
            #       B0/B1 [P,512]  = o2 q0/q1 -> o1 group rotation
            #       C0/C1 [P,512]  = transpose staging -> o2 q2/q3 -> epi rot
            sA = [ps.tile([P, PW], f32, tag="A0", name="sA0"),
                  ps.tile([P, PW], f32, tag="A1", name="sA1")]
            oB = [ps.tile([P, 512], f32, tag="B0", name="oB0"),
                  ps.tile([P, 512], f32, tag="B1", name="oB1")]
            oC = [ps.tile([P, 512], f32, tag="C0", name="oC0"),
                  ps.tile([P, 512], f32, tag="C1", name="oC1")]

            make_identity(nc, ident[:])

            # ---- input DMA (Sync queue; ET pair issues follow) ----
            nc.sync.dma_start(y_sb[:, 0:4], y_dv[:, 0:4])
            nc.sync.dma_start(x_sb[:, 0:4], x_dv[:, 0:4])
            nc.sync.dma_start(y_sb[:, 4:8], y_dv[:, 4:8])
            nc.sync.dma_start(x_sb[:, 4:8], x_dv[:, 4:8])
            nc.sync.dma_start(y_sb[:, 8:16], y_dv[:, 8:16])
            nc.sync.dma_start(x_sb[:, 8:16], x_dv[:, 8:16])

            # ---- head: PE transposes (warmup + exact f32 xT/yT) ----
            ntr = [0]

            def block_T(which, b, eng):
                v_sb, vT = (x_sb, xT) if which == "x" else (y_sb, yT)
                k = ntr[0] % 2
                ntr[0] += 1
                tr = nc.tensor.transpose(oC[k][:, 0:P], v_sb[:, b, :], ident[:])
                nc.vector.tensor_copy(vT[:, b, :], oC[k][:, 0:P])
                return tr

            for which, b in [("y", 0), ("y", 1), ("y", 2), ("y", 3), ("x", 0)]:
                block_T(which, b, nc.scalar)
            for which, b in [("y", 4), ("y", 5), ("y", 6), ("y", 7),
                             ("x", 1), ("x", 2), ("x", 3)]:
                block_T(which, b, nc.vector)
            rest_list = ([("x", b) for b in range(4, 16)]
                         + [("y", b) for b in range(8, 16)])

            nc.vector.tensor_copy(x_hi[:, 0:8], x_sb[:, 0:8])

            yT_f = yT[:].rearrange("p b d -> p (b d)")
            E_t = [E0, E1]

            def et_one(ct, i):
                # transpose E row-block i (panel ct): baseline-identical call
                nc.sync.dma_start_transpose(
                    ET[:, ct * PB:(ct + 1) * PB, i * P:(i + 1) * P],
                    E_t[ct][:, i, :])

            def o1_mov(tb, q):
                return ET[:, tb, q * 512:(q + 1) * 512]

            # ---- panel 0 ----
            for i in range(NB):
                slot = sA[i % 2][:]
                nc.tensor.matmul(slot[:, 0:512], xT[:, i, :],
                                 yT_f[:, 0:512], start=True, stop=True)
                sm = nc.tensor.matmul(slot[:, 512:1024], xT[:, i, :],
                                      yT_f[:, 512:1024], start=True, stop=True)
                if i >= 2:
                    for _ in range(2):
                        if rest_list:
                            tr = block_T(*rest_list.pop(0), nc.vector)
                            add_dep_helper(tr.ins, sm.ins, sync=False,
                                           reason="keep transpose at slot")
                    j = i - 2
                    for q in range(2):
                        om = nc.tensor.matmul(oB[q][:], x_hi[:, j, :],
                                              E0[:, j, q * 512:(q + 1) * 512],
                                              start=(j == 0), stop=(j == NB - 1))
                        if q == 0:
                            add_dep_helper(om.ins, sm.ins, sync=False,
                                           reason="keep o2 at slot")
                nc.scalar.activation(E0[:, i, :], slot, Exp,
                                     accum_out=l1p[:, i, 0:1])
                et_one(0, i)
                if i == 4:
                    nc.vector.tensor_copy(x_hi[:, 8:16], x_sb[:, 8:16])
                if i == 6:
                    nc.vector.tensor_copy(y_hi[:, 0:8], y_sb[:, 0:8])
                if i == 8:
                    nc.vector.tensor_copy(y_hi[:, 8:16], y_sb[:, 8:16])
            for j in (14, 15):
                for q in range(2):
                    nc.tensor.matmul(oB[q][:], x_hi[:, j, :],
                                     E0[:, j, q * 512:(q + 1) * 512],
                                     start=False, stop=(j == NB - 1))

            # ---- panel 1 ----
            for i in range(NB):
                slot = sA[i % 2][:]
                nc.tensor.matmul(slot[:, 0:512], xT[:, i, :],
                                 yT_f[:, 1024:1536], start=True, stop=True)
                sm = nc.tensor.matmul(slot[:, 512:1024], xT[:, i, :],
                                      yT_f[:, 1536:2048], start=True, stop=True)
                if i == 0:
                    # drain o2 cols 0:1024; B banks go to o1 groups
                    nc.vector.tensor_copy(o2T[:, 0:512], oB[0][:])
                    nc.vector.tensor_copy(o2T[:, 512:1024], oB[1][:])
                # o1 first half: group q = i//4 accumulates t-blocks 0..7
                q = i // 4
                for z in range(2):
                    tb = 2 * (i % 4) + z
                    o1m = nc.tensor.matmul(oB[q % 2][:], y_hi[:, tb, :],
                                           o1_mov(tb, q),
                                           start=(tb == 0), stop=(tb == PB - 1))
                    if z == 0:
                        add_dep_helper(o1m.ins, sm.ins, sync=False,
                                       reason="keep o1 at slot")
                if i % 4 == 0 and i >= 4:
                    # previous o1 group finished last iter -> drain it
                    pq = q - 1
                    nc.vector.tensor_copy(o1T[:, pq * 512:(pq + 1) * 512],
                                          oB[pq % 2][:])
                if i >= 2:
                    j = i - 2
                    for qq in range(2):
                        om = nc.tensor.matmul(oC[qq][:], x_hi[:, j, :],
                                              E1[:, j, qq * 512:(qq + 1) * 512],
                                              start=(j == 0), stop=(j == NB - 1))
                        if qq == 0:
                            add_dep_helper(om.ins, sm.ins, sync=False,
                                           reason="keep o2 at slot")
                nc.scalar.activation(E1[:, i, :], slot, Exp,
                                     accum_out=l1p[:, i, 1:2])
                et_one(1, i)
                # l2 for t-blocks 0..7 (full colsum, one fused op each)
                if i % 2 == 1:
                    tb = (i - 1) // 2
                    nc.vector.tensor_reduce(l2[:, tb:tb + 1], ET[:, tb, :],
                                            axis=AX, op=ADD)
            # drain o1 group 3
            nc.vector.tensor_copy(o1T[:, 1536:2048], oB[1][:])
            for j in (14, 15):
                for q in range(2):
                    nc.tensor.matmul(oC[q][:], x_hi[:, j, :],
                                     E1[:, j, q * 512:(q + 1) * 512],
                                     start=False, stop=(j == NB - 1))

            # ---- final: o1 second half + all epilogues ----
            nc.vector.tensor_reduce(l1[:], l1p[:], axis=AX, op=ADD)
            nc.vector.reciprocal(r1[:], l1[:])
            nc.vector.reciprocal(r2[:, 0:8], l2[:, 0:8])
            nc.scalar.copy(o2T[:, 1024:1536], oC[0][:])
            nc.scalar.copy(o2T[:, 1536:2048], oC[1][:])

            erot_tags = [sA[0], sA[1], oC[0], oC[1]]
            erot = [0]
            st1g = [None] * 4
            st2g = [None] * 4

            def epi1_step(j, pin=None):
                # a1 block j: transpose o1T col-block, * r1 * x (DVE stt)
                g = j // 4
                if j % 4 == 0:
                    st1g[g] = stg.tile([P, 4, D], f32, tag="st1",
                                       name=f"st1_{g}")
                bank = erot_tags[erot[0] % 4]
                erot[0] += 1
                tr = nc.tensor.transpose(bank[:, 0:P],
                                         o1T[:, j * P:(j + 1) * P], ident[:])
                if pin is not None:
                    add_dep_helper(tr.ins, pin.ins, sync=False,
                                   reason="epi1 slot")
                nc.vector.scalar_tensor_tensor(st1g[g][:, j % 4, :],
                                               bank[:, 0:P], r1[:, j:j + 1],
                                               x_sb[:, j, :], op0=MUL, op1=MUL)
                if j % 4 == 3:
                    nc.sync.dma_start(out_dv[:, g * 4:(g + 1) * 4, 0:D],
                                        st1g[g][:])

            def epi2_step(j, pin=None, dve=False):
                # a2 block j: transpose o2T col-block, * r2 * y
                g = j // 4
                if j % 4 == 0:
                    st2g[g] = stg.tile([P, 4, D], f32, tag="st2",
                                       name=f"st2_{g}")
                bank = erot_tags[erot[0] % 4]
                erot[0] += 1
                tr = nc.tensor.transpose(bank[:, 0:P],
                                         o2T[:, j * P:(j + 1) * P], ident[:])
                if pin is not None:
                    add_dep_helper(tr.ins, pin.ins, sync=False,
                                   reason="epi2 slot")
                nc.scalar.activation(sc2[:, j % PB, :], bank[:, 0:P],
                                     Copy, scale=r2[:, j:j + 1])
                nc.gpsimd.tensor_tensor(st2g[g][:, j % 4, :],
                                        sc2[:, j % PB, :], y_sb[:, j, :],
                                        op=MUL)
                if j % 4 == 3:
                    nc.sync.dma_start(out_dv[:, g * 4:(g + 1) * 4, D:2 * D],
                                        st2g[g][:])

            # o1 second half: 4 groups of 8 (t-blocks 8..15), epis interleaved
            for q in range(4):
                last = None
                for tb in range(PB, NB):
                    last = nc.tensor.matmul(oB[q % 2][:], y_hi[:, tb, :],
                                            o1_mov(tb, q),
                                            start=(tb == PB), stop=(tb == NB - 1))
                    if tb - PB < 4 and q > 0:
                        # previous group's epi1 blocks
                        epi1_step(4 * (q - 1) + (tb - PB), pin=last)
                    elif tb - PB in (4, 5) and q > 0:
                        epi2_step(2 * (q - 1) + (tb - PB - 4), pin=last)
                # drain-add: o1T cols q = first-half + second-half
                nc.vector.tensor_tensor(o1T[:, q * 512:(q + 1) * 512],
                                        o1T[:, q * 512:(q + 1) * 512],
                                        oB[q % 2][:], op=ADD)
            for k in range(4):
                epi1_step(12 + k)
            for k in range(6, 8):
                epi2_step(k)
            # colsums for t-blocks 8..15: split across ACT and DVE
            for tbg in range(PB, PB + 4):
                nc.scalar.activation(scr[:], ET[:, tbg, :], Copy,
                                     accum_out=l2[:, tbg:tbg + 1])
            for tbg in range(PB + 4, NB):
                nc.vector.tensor_reduce(l2[:, tbg:tbg + 1], ET[:, tbg, :],
                                        axis=AX, op=ADD)
            nc.vector.reciprocal(r2[:, 8:16], l2[:, 8:16])
            for j in range(8, 16):
                epi2_step(j)

    nc.compile()
    return nc


def _get_nc():
    global _NC_CACHE
    if _NC_CACHE is None:
        nc = bacc.Bacc("TRN2", target_bir_lowering=False, debug=False,
                       num_devices=B)
        _NC_CACHE = _build_program(nc)
    return _NC_CACHE


def kernel(x, y):
    global LAST_EXEC_NS
    nc = _get_nc()
    x = np.asarray(x, dtype=np.float32)
    y = np.asarray(y, dtype=np.float32)
    in_maps = [
        {"x": np.ascontiguousarray(x[b]), "y": np.ascontiguousarray(y[b])}
        for b in range(B)
    ]
    trace = bool(int(os.environ.get("KERNEL_TRACE", "0")))
    res = run_bass_kernel_spmd(nc, in_maps, list(range(B)), trace=trace)
    LAST_EXEC_NS = res.exec_time_ns
    return np.stack([res.results[b]["out"] for b in range(B)], axis=0)
